# revision 13
# baseline (speedup 1.0000x reference)
# DenseGATv2Conv Trainium2 kernel.
#
# Math (per batch b):
#   xl = x @ W_l + b_l ; xr = x @ W_r + b_r            [N, H*C]
#   alpha[i,j,h] = sum_c att[h,c] * leaky_relu(xl[j,hc] + xr[i,hc], 0.2)
#   S = softmax_j(alpha masked by adj(+self loops))
#   out[i,hc] = sum_j S[i,j,h] * xr[j,hc] + bias
#
# Key identity used on device:
#   leaky_relu(z) = 0.2*z + 0.8*relu(z)
#   alpha[i,j,h] = 0.2*sl[j,h] + 0.2*sr[i,h] + 0.8*sum_c att[h,c]*relu(xl[j,hc]+xr[i,hc])
# where sl = xl @ att_blk, sr = xr @ att_blk are rank-1 in the (i,j) plane.
# In softmax over j the exp(0.2*sr[i,h]) factor cancels; exp(0.2*sl[j,h]) is
# folded multiplicatively into the aggregation operand.
#
# Device scheme (v2): for each pair of destination rows (2 per "pair", 16
# pairs = 32 dest rows per "super"), DVE/Act/Pool build
#   rp[(d,hc), j] = relu(xl[j,hc] + xr[i_d,hc])         [128, N] f16
# Then the score contraction runs with rp as the matmul STATIONARY operand
# and a tiny [128, 8] att operand moving, so the result lands in PSUM
# already transposed: pst[j, (pair,d,h)].  The adjacency mask is folded in
# as an extra accumulating matmul that adds -30 to masked entries
# (adjm30 = 30*(adj-1) in {0,-30} f16, stationary; selection matrix E30
# moving), so a single exp() per super yields masked scores directly
# (exp(a-30) underflows f16 to 0).  The softmax denominator is computed in
# the aggregation matmul via an extra ones-like column (esl factor).
#
# Sharding: 8 cores = (batch b in 0..1) x (4 blocks of 256 destination rows).

import numpy as np

B, N, F, H, C = 2, 1024, 128, 4, 16
HC = H * C
NCORES = 8
NI = 256          # destination rows per core
NPAIR = NI // 2   # 128 pairs of destination rows
NSUP = 8          # supers of 16 pairs (32 dest rows) each
NEG = 0.2
MASKVAL = 30.0

_CACHE = {}
LAST_RESULTS = None

# engine assignment for the 16 rp builds of each super:
# 'd' = DVE tensor_scalar, 'a' = Act activation, 'p' = Pool tensor_scalar
RP_SCHED = "ddddddddddddaapp"

# blob column layout (f32 [128, BLOB_COLS]):
#   xb      [128, 8*128]   x[b] nodes, node k*128+p at cols k*128..k*128+128? no:
#                          xin[p, k*128+f] = x[k*128+p, f]
#   xis     [128, 2*128]   dest-row slice, same layout
#   wl      [128, 64]
#   wr      [128, 64]
#   ident   [128, 128]
#   biasb   [128, 64]
#   att2p8  [128, 8]
#   blp2    [128, 1]       b_l tiled twice
#   brp2    [128, 1]       b_r tiled twice
#   attb    [128, 4]       att_blk (rows 64:128 zero)
#   e30     [128, 128]     selection matrix for mask matmul
_OFF = {}
_c = 0
for _nm, _w in [("ident", 128), ("xb", 8 * F), ("xis", 2 * F), ("wl", HC),
                ("wr", HC), ("wrab", HC + H), ("biasb", HC), ("att2p8", 8),
                ("blp2", 1), ("brp2", 1), ("e30", 128)]:
    _OFF[_nm] = _c
    _c += _w
BLOB_COLS = _c


def _build_program(debug=False):
    import concourse.bass as bass
    import concourse.mybir as mybir
    import concourse.tile as tile
    from concourse import bacc

    f32 = mybir.dt.float32
    f32r = mybir.dt.float32r
    f16 = mybir.dt.float16

    nc = bacc.Bacc(
        "TRN2",
        target_bir_lowering=False,
        debug=False,
        enable_asserts=False,
        num_devices=NCORES,
    )

    # ---- DRAM I/O ----
    blob = nc.dram_tensor("blob", [128, BLOB_COLS], f32, kind="ExternalInput").ap()
    adjs = nc.dram_tensor("adjs", [128, 2 * N], f16, kind="ExternalInput").ap()
    out = nc.dram_tensor("out", [NI, HC], f32, kind="ExternalOutput").ap()

    with tile.TileContext(nc) as tc:
        _body(tc, nc, mybir, bass, f32, f32r, f16, blob, adjs, out)

    nc.compile()
    return nc


def _body(tc, nc, mybir, bass, f32, f32r, f16, blob, adjs, out):
    from contextlib import ExitStack
    Alu = mybir.AluOpType
    Act = mybir.ActivationFunctionType
    ctx = ExitStack()
    with ctx:
        consts = ctx.enter_context(tc.tile_pool(name="consts", bufs=1))
        work = ctx.enter_context(tc.tile_pool(name="work", bufs=1))
        rp_pool = ctx.enter_context(tc.tile_pool(name="rp", bufs=2))
        outp = ctx.enter_context(tc.tile_pool(name="outp", bufs=2))
        psg = ctx.enter_context(tc.tile_pool(name="psg", bufs=1, space="PSUM"))
        pss = ctx.enter_context(tc.tile_pool(name="pss", bufs=2, space="PSUM"))
        pst = ctx.enter_context(tc.tile_pool(name="pst", bufs=2, space="PSUM"))

        dma = nc.sync.dma_start

        # ---------- load inputs ----------
        blob_t = consts.tile([128, BLOB_COLS], f32, tag="blob")
        c1 = _OFF["xb"]
        c2 = _OFF["xis"]
        dma(blob_t[:, 0:c1], blob[:, 0:c1])
        dma(blob_t[:, c1:c2], blob[:, c1:c2])
        dma(blob_t[:, c2:], blob[:, c2:])
        adjm = consts.tile([128, 2 * N], f16, tag="adjm")   # 30*(adj-1), [i128][ib*N+j]
        dma(adjm[:], adjs)

        def bv(nm, w):
            return blob_t[:, _OFF[nm]:_OFF[nm] + w]

        xin = bv("xb", 8 * F)
        xis_t = bv("xis", 2 * F)
        id_t = bv("ident", 128)
        biasb_t = bv("biasb", HC)
        blp_t = blob_t[0:HC, _OFF["blp2"]:_OFF["blp2"] + 1]
        brp_t = blob_t[0:HC, _OFF["brp2"]:_OFF["brp2"] + 1]

        # f32r / f16 copies of small constants
        wl_r = consts.tile([F, HC], f32r, tag="wlr")
        wr_r = consts.tile([F, HC], f32r, tag="wrr")
        wrab_r = consts.tile([F, HC + H], f32r, tag="wrabr")
        att8_r = consts.tile([F, 8], f16, tag="att8r")
        e30_16 = consts.tile([128, 128], f16, tag="e30")
        nc.vector.tensor_copy(wl_r[:], bv("wl", HC))
        nc.vector.tensor_copy(wr_r[:], bv("wr", HC))
        nc.gpsimd.tensor_copy(wrab_r[:], bv("wrab", HC + H))
        nc.vector.tensor_copy(att8_r[:], bv("att2p8", 8))
        nc.gpsimd.tensor_copy(e30_16[:], bv("e30", 128))

        # ---------- xisT / xrsT / xrp (critical path to rp) ----------
        xisT = consts.tile([F, NI], f32r, tag="xisT")
        for k in range(2):
            pt = pst.tile([128, 128], f32, tag="pt")
            nc.tensor.transpose(pt[:], xis_t[:, k * F:(k + 1) * F], id_t)
            nc.vector.tensor_copy(xisT[:, k * 128:(k + 1) * 128], pt[:])
        xrsT = consts.tile([HC, NI], f32, tag="xrsT")
        g3 = psg.tile([128, N], f32, tag="g")
        pj3 = g3[0:HC, 0:NI]
        nc.tensor.matmul(pj3, wr_r[:], xisT[:], start=True, stop=True)
        nc.scalar.activation(xrsT[:], pj3, Act.Identity,
                             bias=brp_t, scale=1.0)
        xrp = consts.tile([128, NPAIR], f32, tag="xrp")
        ev = xrsT[:].rearrange("p (a two) -> p a two", two=2)
        nc.vector.tensor_copy(xrp[0:HC, :], ev[:, :, 0])
        nc.vector.tensor_copy(xrp[HC:128, :], ev[:, :, 1])

        # ---------- xT + xl2T ----------
        xT = consts.tile([F, N], f32r, tag="xT")              # [f, node]
        for k in range(8):
            pt = pst.tile([128, 128], f32, tag="pt")
            nc.tensor.transpose(pt[:], xin[:, k * F:(k + 1) * F], id_t)
            if k % 2 == 0:
                nc.vector.tensor_copy(xT[:, k * 128:(k + 1) * 128], pt[:])
            else:
                nc.scalar.copy(xT[:, k * 128:(k + 1) * 128], pt[:])
        xl2T = consts.tile([128, N], f16, tag="xl2T")
        gp = psg.tile([128, N], f32, tag="g")
        pj = gp[0:HC, :]
        for half in range(2):
            s = slice(half * 512, (half + 1) * 512)
            nc.tensor.matmul(pj[:, s], wl_r[:], xT[:, s], start=True, stop=True)
        nc.scalar.activation(xl2T[0:HC, :], pj, Act.Identity,
                             bias=blp_t, scale=1.0)
        nc.vector.tensor_copy(xl2T[HC:128, :], xl2T[0:HC, :])

        # ---------- xr_mod: aggregation moving operand (natural layout) ----------
        # xr_mod[j, k*68 + h*17 + c] = xr_nob[j,hc]*esl[j,h] (c<16); c=16: esl[j,h]
        # xr_nob excludes b_r (folded into the output bias since sum_j Sbar = 1);
        # esl = exp(0.2 * x @ (W_l @ att_blk)) (the b_l@att_blk factor cancels).
        xr_mod = consts.tile([128, 8 * 68], f16, tag="xrmod")
        xrmv = xr_mod[:].rearrange("p (k h c) -> p k h c", k=8, h=H)

        def build_xr_mod_k(k):
            pkt = pst.tile([128, 128], f32, tag="pt", name="pk")
            pk = pkt[:, 0:HC + H]
            nc.tensor.matmul(pk, xT[:, k * 128:(k + 1) * 128], wrab_r[:],
                             start=True, stop=True)
            esl4 = work.tile([128, 8 * H], f16, tag="esl4", name="esl4")
            xr16 = work.tile([128, 8 * HC], f16, tag="xr16", name="xr16")
            nc.scalar.activation(esl4[:, k * H:(k + 1) * H],
                                 pkt[:, HC:HC + H], Act.Exp, scale=NEG)
            nc.scalar.copy(xr16[:, k * HC:(k + 1) * HC], pkt[:, 0:HC])
            nc.gpsimd.tensor_copy(xrmv[:, k, :, 16], esl4[:, k * H:(k + 1) * H])
            rep = esl4[:, k * H:(k + 1) * H].rearrange(
                "p (h one) -> p h one", one=1).broadcast_to([128, H, 16])
            srcx = xr16[:, k * HC:(k + 1) * HC].rearrange("p (h c) -> p h c", h=H)
            nc.gpsimd.tensor_tensor(xrmv[:, k, :, 0:16], srcx, rep, Alu.mult)

        # ---------- main streaming loop ----------
        # st_all[j, k*1024 + s*128 + a*4 + h], a = local dest (2*p+d) in super s
        # (k-major so the aggregation lhsT slice has ONE contiguous free dim)
        st_all = consts.tile([128, NSUP * N], f16, tag="stall")
        stv = st_all[:].rearrange("p (k s a h) -> p k s a h", k=8, s=NSUP, h=H)

        def super_iter(sup):
            ib, s4 = sup // 4, sup % 4
            if 1 <= sup <= 4:
                build_xr_mod_k(2 * (sup - 1))
                build_xr_mod_k(2 * (sup - 1) + 1)
            rp = rp_pool.tile([128, 16, 1024], f16, tag="rp")
            for p in range(16):
                gp = sup * 16 + p
                kind = RP_SCHED[p]
                if kind == "a":
                    nc.scalar.activation(rp[:, p, :], xl2T[:], Act.Relu,
                                         bias=xrp[:, gp:gp + 1], scale=1.0)
                elif kind == "p":
                    nc.gpsimd.tensor_scalar(rp[:, p, :], xl2T[:],
                                            xrp[:, gp:gp + 1],
                                            0.0, Alu.add, Alu.max)
                else:
                    nc.vector.tensor_scalar(rp[:, p, :], xl2T[:],
                                            xrp[:, gp:gp + 1],
                                            0.0, Alu.add, Alu.max)
            ps = pss.tile([128, 8, 128], f32, tag="sc")
            for k in range(8):
                # mask matmul: adds 30*(adj-1) (0 or -30) to every (j, a, h)
                nc.tensor.matmul(ps[:, k, :],
                                 adjm[s4 * 32:(s4 + 1) * 32,
                                      ib * N + k * 128: ib * N + (k + 1) * 128],
                                 e30_16[s4 * 32:(s4 + 1) * 32, :],
                                 start=True, stop=False, skip_group_check=True,
                                 tile_position=(s4 * 32, 0))
                for p in range(16):
                    nc.tensor.matmul(ps[:, k, p * 8:(p + 1) * 8],
                                     rp[:, p, k * 128:(k + 1) * 128],
                                     att8_r[:],
                                     start=False, stop=(p == 15),
                                     skip_group_check=True)
            nc.scalar.activation(stv[:, :, sup, :, :],
                                 ps[:].rearrange("p k (a h) -> p k a h", h=H), Act.Exp)

        # ---------- aggregation ----------
        # agg psum: one [128, 1024] tile per ib; head h uses cols h*32..h*32+17
        agg_ga = {}

        def agg_mms(ib, s0, ns, first):
            # accumulate supers [ib*4+s0, ib*4+s0+ns) -> dest rows s0*32..(s0+ns)*32
            if first:
                agg_ga[ib] = psg.tile([128, N], f32, tag="g", name="agg")
            ga = agg_ga[ib]
            for h in range(H):
                o = ga[s0 * 32:(s0 + ns) * 32, h * 32:h * 32 + 17]
                for k in range(8):
                    lhs = stv[:, k, ib * 4 + s0:ib * 4 + s0 + ns, :, h]
                    rhs = xr_mod[:, k * 68 + h * 17: k * 68 + (h + 1) * 17]
                    nc.tensor.matmul(o, lhs, rhs,
                                     start=(k == 0), stop=(k == 7),
                                     skip_group_check=True,
                                     tile_position=(0, s0 * 32))

        ofs = {}

        def agg_div(ib, r0, r1, first):
            ga = agg_ga[ib]
            if first:
                ofs[ib] = (outp.tile([128, HC], f32, tag="outf", name="outf"),
                           outp.tile([128, HC], f32, tag="outf2", name="outf2"))
            out_f, out_f2 = ofs[ib]
            for h in range(H):
                rz = work.tile([128, 1], f32, tag="rz", name="rz")
                nc.vector.reciprocal(rz[r0:r1, :],
                                     ga[r0:r1, h * 32 + 16:h * 32 + 17])
                nc.vector.tensor_scalar(out_f[r0:r1, h * 16:(h + 1) * 16],
                                        ga[r0:r1, h * 32:h * 32 + 16],
                                        rz[r0:r1, 0:1], None, Alu.mult)
            nc.vector.tensor_add(out_f2[r0:r1, :], out_f[r0:r1, :],
                                 biasb_t[r0:r1, :] if hasattr(biasb_t, 'rearrange') else biasb_t)
            dma(out[ib * 128 + r0:ib * 128 + r1, :], out_f2[r0:r1, :])

        for sup in range(NSUP):
            super_iter(sup)
            if sup == 5:
                agg_mms(0, 0, 4, True)
            elif sup == 6:
                agg_div(0, 0, 128, True)
                agg_mms(1, 0, 3, True)
        agg_div(1, 0, 96, True)
        agg_mms(1, 3, 1, False)
        agg_div(1, 96, 128, False)


def _get_program():
    if "nc" not in _CACHE:
        _CACHE["nc"] = _build_program()
    return _CACHE["nc"]


def kernel(x, adj, W_l, b_l, W_r, b_r, att, bias):
    global LAST_RESULTS
    from concourse.bass_utils import run_bass_kernel_spmd

    x = np.ascontiguousarray(np.asarray(x, dtype=np.float32))
    adj = np.ascontiguousarray(np.asarray(adj, dtype=np.float32))
    W_l = np.asarray(W_l, dtype=np.float32)
    b_l = np.asarray(b_l, dtype=np.float32)
    W_r = np.asarray(W_r, dtype=np.float32)
    b_r = np.asarray(b_r, dtype=np.float32)
    att = np.asarray(att, dtype=np.float32)
    bias = np.asarray(bias, dtype=np.float32)

    # host-side constant prep
    att2p8 = np.zeros((F, 8), np.float32)        # 0.8 * att, block diagonal x2
    for d in range(2):
        for h in range(H):
            att2p8[d * HC + h * C:(d * HC + (h + 1) * C), d * H + h] = 0.8 * att[h]
    attblk = np.zeros((HC, H), np.float32)       # att_blk
    for h in range(H):
        attblk[h * C:(h + 1) * C, h] = att[h]
    wrab = np.concatenate([W_r, W_l @ attblk], axis=1)   # [F, 68]
    e30 = np.zeros((128, 128), np.float32)       # E30[r, a*4+h] = (r%32 == a)
    r = np.arange(128)
    for a in range(32):
        for h in range(H):
            e30[r[r % 32 == a], a * 4 + h] = 1.0

    blob = np.zeros((128, BLOB_COLS), np.float32)

    def put(nm, arr):
        w = arr.shape[1]
        blob[:arr.shape[0], _OFF[nm]:_OFF[nm] + w] = arr

    put("wl", W_l)
    put("wr", W_r)
    put("wrab", wrab)
    put("ident", np.eye(128, dtype=np.float32))
    put("biasb", np.broadcast_to(bias + b_r, (128, HC)))
    put("att2p8", att2p8)
    put("blp2", np.tile(b_l, 2).reshape(128, 1))
    put("brp2", np.tile(b_r, 2).reshape(128, 1))
    put("e30", e30)

    in_maps = []
    for core in range(NCORES):
        b, blk = core // 4, core % 4
        i0 = blk * NI
        cblob = blob.copy()
        # xin[p, k*128+f] = x[b, k*128+p, f]
        cblob[:, _OFF["xb"]:_OFF["xb"] + 8 * F] = (
            x[b].reshape(8, 128, F).transpose(1, 0, 2).reshape(128, 8 * F))
        cblob[:, _OFF["xis"]:_OFF["xis"] + 2 * F] = (
            x[b, i0:i0 + NI].reshape(2, 128, F).transpose(1, 0, 2).reshape(128, 2 * F))
        adjsl = adj[b, i0:i0 + NI, :].copy()
        adjsl[np.arange(NI), i0 + np.arange(NI)] = 1.0   # self loops
        adjm30 = (MASKVAL * (adjsl - 1.0)).astype(np.float16)
        # adjm[p, ib*N + j] = adjm30[ib*128 + p, j]
        adjm = adjm30.reshape(2, 128, N).transpose(1, 0, 2).reshape(128, 2 * N).copy()
        in_maps.append({"blob": cblob, "adjs": adjm})

    nc = _get_program()
    res = run_bass_kernel_spmd(nc, in_maps, core_ids=list(range(NCORES)))
    LAST_RESULTS = res
    outp = np.zeros((B, N, HC), np.float32)
    for core in range(NCORES):
        b, blk = core // 4, core % 4
        outp[b, blk * NI:(blk + 1) * NI, :] = res.results[core]["out"]
    return outp


# revision 14
# speedup vs baseline: 1.0264x; 1.0264x over previous
# DenseGATv2Conv Trainium2 kernel.
#
# Math (per batch b):
#   xl = x @ W_l + b_l ; xr = x @ W_r + b_r            [N, H*C]
#   alpha[i,j,h] = sum_c att[h,c] * leaky_relu(xl[j,hc] + xr[i,hc], 0.2)
#   S = softmax_j(alpha masked by adj(+self loops))
#   out[i,hc] = sum_j S[i,j,h] * xr[j,hc] + bias
#
# Key identity used on device:
#   leaky_relu(z) = 0.2*z + 0.8*relu(z)
#   alpha[i,j,h] = 0.2*sl[j,h] + 0.2*sr[i,h] + 0.8*sum_c att[h,c]*relu(xl[j,hc]+xr[i,hc])
# where sl = xl @ att_blk, sr = xr @ att_blk are rank-1 in the (i,j) plane.
# In softmax over j the exp(0.2*sr[i,h]) factor cancels; exp(0.2*sl[j,h]) is
# folded multiplicatively into the aggregation operand.
#
# Device scheme (v2): for each pair of destination rows (2 per "pair", 16
# pairs = 32 dest rows per "super"), DVE/Act/Pool build
#   rp[(d,hc), j] = relu(xl[j,hc] + xr[i_d,hc])         [128, N] f16
# Then the score contraction runs with rp as the matmul STATIONARY operand
# and a tiny [128, 8] att operand moving, so the result lands in PSUM
# already transposed: pst[j, (pair,d,h)].  The adjacency mask is folded in
# as an extra accumulating matmul that adds -30 to masked entries
# (adjm30 = 30*(adj-1) in {0,-30} f16, stationary; selection matrix E30
# moving), so a single exp() per super yields masked scores directly
# (exp(a-30) underflows f16 to 0).  The softmax denominator is computed in
# the aggregation matmul via an extra ones-like column (esl factor).
#
# Sharding: 8 cores = (batch b in 0..1) x (4 blocks of 256 destination rows).

import numpy as np

B, N, F, H, C = 2, 1024, 128, 4, 16
HC = H * C
NCORES = 8
NI = 256          # destination rows per core
NPAIR = NI // 2   # 128 pairs of destination rows
NSUP = 8          # supers of 16 pairs (32 dest rows) each
NEG = 0.2
MASKVAL = 30.0

_CACHE = {}
LAST_RESULTS = None

# engine assignment for the 16 rp builds of each super:
# 'd' = DVE tensor_scalar, 'a' = Act activation, 'p' = Pool tensor_scalar
RP_SCHED = "ddddddddddddaapp"

# blob column layout (f32 [128, BLOB_COLS]):
#   xb      [128, 8*128]   x[b] nodes, node k*128+p at cols k*128..k*128+128? no:
#                          xin[p, k*128+f] = x[k*128+p, f]
#   xis     [128, 2*128]   dest-row slice, same layout
#   wl      [128, 64]
#   wr      [128, 64]
#   ident   [128, 128]
#   biasb   [128, 64]
#   att2p8  [128, 8]
#   blp2    [128, 1]       b_l tiled twice
#   brp2    [128, 1]       b_r tiled twice
#   attb    [128, 4]       att_blk (rows 64:128 zero)
#   e30     [128, 128]     selection matrix for mask matmul
_OFF = {}
_c = 0
for _nm, _w in [("ident", 128), ("xis", 2 * F), ("wl", HC), ("wr", HC),
                ("wrab", HC + H), ("biasb", HC), ("att2p8", 8), ("blp2", 1),
                ("brp2", 1), ("xb", 8 * F), ("e30", 128)]:
    _OFF[_nm] = _c
    _c += _w
BLOB_COLS = _c


def _build_program(debug=False):
    import concourse.bass as bass
    import concourse.mybir as mybir
    import concourse.tile as tile
    from concourse import bacc

    f32 = mybir.dt.float32
    f32r = mybir.dt.float32r
    f16 = mybir.dt.float16

    nc = bacc.Bacc(
        "TRN2",
        target_bir_lowering=False,
        debug=False,
        enable_asserts=False,
        num_devices=NCORES,
    )

    # ---- DRAM I/O ----
    blob = nc.dram_tensor("blob", [128, BLOB_COLS], f32, kind="ExternalInput").ap()
    adjs = nc.dram_tensor("adjs", [128, 2 * N], f16, kind="ExternalInput").ap()
    out = nc.dram_tensor("out", [NI, HC], f32, kind="ExternalOutput").ap()

    with tile.TileContext(nc) as tc:
        _body(tc, nc, mybir, bass, f32, f32r, f16, blob, adjs, out)

    nc.compile()
    return nc


def _body(tc, nc, mybir, bass, f32, f32r, f16, blob, adjs, out):
    from contextlib import ExitStack
    Alu = mybir.AluOpType
    Act = mybir.ActivationFunctionType
    ctx = ExitStack()
    with ctx:
        consts = ctx.enter_context(tc.tile_pool(name="consts", bufs=1))
        work = ctx.enter_context(tc.tile_pool(name="work", bufs=1))
        rp_pool = ctx.enter_context(tc.tile_pool(name="rp", bufs=2))
        outp = ctx.enter_context(tc.tile_pool(name="outp", bufs=2))
        psg = ctx.enter_context(tc.tile_pool(name="psg", bufs=1, space="PSUM"))
        pss = ctx.enter_context(tc.tile_pool(name="pss", bufs=2, space="PSUM"))
        pst = ctx.enter_context(tc.tile_pool(name="pst", bufs=2, space="PSUM"))

        dma = nc.sync.dma_start

        # ---------- load inputs ----------
        blob_t = consts.tile([128, BLOB_COLS], f32, tag="blob")
        c1 = _OFF["xb"]
        c2 = _OFF["e30"]
        dma(blob_t[:, 0:c1], blob[:, 0:c1])
        dma(blob_t[:, c1:c2], blob[:, c1:c2])
        dma(blob_t[:, c2:], blob[:, c2:])
        adjm = consts.tile([128, 2 * N], f16, tag="adjm")   # 30*(adj-1), [i128][ib*N+j]
        dma(adjm[:], adjs)

        def bv(nm, w):
            return blob_t[:, _OFF[nm]:_OFF[nm] + w]

        xin = bv("xb", 8 * F)
        xis_t = bv("xis", 2 * F)
        id_t = bv("ident", 128)
        biasb_t = bv("biasb", HC)
        blp_t = blob_t[0:HC, _OFF["blp2"]:_OFF["blp2"] + 1]
        brp_t = blob_t[0:HC, _OFF["brp2"]:_OFF["brp2"] + 1]

        # f32r / f16 copies of small constants
        wl_r = consts.tile([F, HC], f32r, tag="wlr")
        wr_r = consts.tile([F, HC], f32r, tag="wrr")
        wrab_r = consts.tile([F, HC + H], f32r, tag="wrabr")
        att8_r = consts.tile([F, 8], f16, tag="att8r")
        e30_16 = consts.tile([128, 128], f16, tag="e30")
        nc.vector.tensor_copy(wl_r[:], bv("wl", HC))
        nc.vector.tensor_copy(wr_r[:], bv("wr", HC))
        nc.gpsimd.tensor_copy(wrab_r[:], bv("wrab", HC + H))
        nc.vector.tensor_copy(att8_r[:], bv("att2p8", 8))
        nc.gpsimd.tensor_copy(e30_16[:], bv("e30", 128))

        # ---------- xisT / xrsT / xrp (critical path to rp) ----------
        xisT = consts.tile([F, NI], f32r, tag="xisT")
        for k in range(2):
            pt = pst.tile([128, 128], f32, tag="pt")
            nc.tensor.transpose(pt[:], xis_t[:, k * F:(k + 1) * F], id_t)
            nc.vector.tensor_copy(xisT[:, k * 128:(k + 1) * 128], pt[:])
        xrsT = consts.tile([HC, NI], f32, tag="xrsT")
        g3 = psg.tile([128, N], f32, tag="g")
        pj3 = g3[0:HC, 0:NI]
        nc.tensor.matmul(pj3, wr_r[:], xisT[:], start=True, stop=True)
        nc.scalar.activation(xrsT[:], pj3, Act.Identity,
                             bias=brp_t, scale=1.0)
        xrp = consts.tile([128, NPAIR], f32, tag="xrp")
        ev = xrsT[:].rearrange("p (a two) -> p a two", two=2)
        nc.vector.tensor_copy(xrp[0:HC, :], ev[:, :, 0])
        nc.vector.tensor_copy(xrp[HC:128, :], ev[:, :, 1])

        # ---------- xT + xl2T ----------
        xT = consts.tile([F, N], f32r, tag="xT")              # [f, node]
        for k in range(8):
            pt = pst.tile([128, 128], f32, tag="pt")
            nc.tensor.transpose(pt[:], xin[:, k * F:(k + 1) * F], id_t)
            if k % 2 == 0:
                nc.vector.tensor_copy(xT[:, k * 128:(k + 1) * 128], pt[:])
            else:
                nc.scalar.copy(xT[:, k * 128:(k + 1) * 128], pt[:])
        xl2T = consts.tile([128, N], f16, tag="xl2T")
        gp = psg.tile([128, N], f32, tag="g")
        pj = gp[0:HC, :]
        for half in range(2):
            s = slice(half * 512, (half + 1) * 512)
            nc.tensor.matmul(pj[:, s], wl_r[:], xT[:, s], start=True, stop=True)
        nc.scalar.activation(xl2T[0:HC, :], pj, Act.Identity,
                             bias=blp_t, scale=1.0)
        nc.vector.tensor_copy(xl2T[HC:128, :], xl2T[0:HC, :])

        # ---------- xr_mod: aggregation moving operand (natural layout) ----------
        # xr_mod[j, k*68 + h*17 + c] = xr_nob[j,hc]*esl[j,h] (c<16); c=16: esl[j,h]
        # xr_nob excludes b_r (folded into the output bias since sum_j Sbar = 1);
        # esl = exp(0.2 * x @ (W_l @ att_blk)) (the b_l@att_blk factor cancels).
        xr_mod = consts.tile([128, 8 * 68], f16, tag="xrmod")
        xrmv = xr_mod[:].rearrange("p (k h c) -> p k h c", k=8, h=H)

        def build_xr_mod_k(k):
            pkt = pst.tile([128, 128], f32, tag="pt", name="pk")
            pk = pkt[:, 0:HC + H]
            nc.tensor.matmul(pk, xT[:, k * 128:(k + 1) * 128], wrab_r[:],
                             start=True, stop=True)
            esl4 = work.tile([128, 8 * H], f16, tag="esl4", name="esl4")
            xr16 = work.tile([128, 8 * HC], f16, tag="xr16", name="xr16")
            nc.scalar.activation(esl4[:, k * H:(k + 1) * H],
                                 pkt[:, HC:HC + H], Act.Exp, scale=NEG)
            nc.scalar.copy(xr16[:, k * HC:(k + 1) * HC], pkt[:, 0:HC])
            nc.gpsimd.tensor_copy(xrmv[:, k, :, 16], esl4[:, k * H:(k + 1) * H])
            rep = esl4[:, k * H:(k + 1) * H].rearrange(
                "p (h one) -> p h one", one=1).broadcast_to([128, H, 16])
            srcx = xr16[:, k * HC:(k + 1) * HC].rearrange("p (h c) -> p h c", h=H)
            nc.gpsimd.tensor_tensor(xrmv[:, k, :, 0:16], srcx, rep, Alu.mult)

        # ---------- main streaming loop ----------
        # st_all[j, k*1024 + s*128 + a*4 + h], a = local dest (2*p+d) in super s
        # (k-major so the aggregation lhsT slice has ONE contiguous free dim)
        st_all = consts.tile([128, NSUP * N], f16, tag="stall")
        stv = st_all[:].rearrange("p (k s a h) -> p k s a h", k=8, s=NSUP, h=H)

        def super_iter(sup):
            ib, s4 = sup // 4, sup % 4
            if 1 <= sup <= 4:
                build_xr_mod_k(2 * (sup - 1))
                build_xr_mod_k(2 * (sup - 1) + 1)
            rp = rp_pool.tile([128, 16, 1024], f16, tag="rp")
            for p in range(16):
                gp = sup * 16 + p
                kind = RP_SCHED[p]
                if kind == "a":
                    nc.scalar.activation(rp[:, p, :], xl2T[:], Act.Relu,
                                         bias=xrp[:, gp:gp + 1], scale=1.0)
                elif kind == "p":
                    nc.gpsimd.tensor_scalar(rp[:, p, :], xl2T[:],
                                            xrp[:, gp:gp + 1],
                                            0.0, Alu.add, Alu.max)
                else:
                    nc.vector.tensor_scalar(rp[:, p, :], xl2T[:],
                                            xrp[:, gp:gp + 1],
                                            0.0, Alu.add, Alu.max)
            ps = pss.tile([128, 8, 128], f32, tag="sc")
            for k in range(8):
                # mask matmul: adds 30*(adj-1) (0 or -30) to every (j, a, h)
                nc.tensor.matmul(ps[:, k, :],
                                 adjm[s4 * 32:(s4 + 1) * 32,
                                      ib * N + k * 128: ib * N + (k + 1) * 128],
                                 e30_16[s4 * 32:(s4 + 1) * 32, :],
                                 start=True, stop=False, skip_group_check=True,
                                 tile_position=(s4 * 32, 0))
                for p in range(16):
                    nc.tensor.matmul(ps[:, k, p * 8:(p + 1) * 8],
                                     rp[:, p, k * 128:(k + 1) * 128],
                                     att8_r[:],
                                     start=False, stop=(p == 15),
                                     skip_group_check=True)
            nc.scalar.activation(stv[:, :, sup, :, :],
                                 ps[:].rearrange("p k (a h) -> p k a h", h=H), Act.Exp)

        # ---------- aggregation ----------
        # agg psum: one [128, 1024] tile per ib; head h uses cols h*32..h*32+17
        agg_ga = {}

        def agg_mms(ib, s0, ns, first):
            # accumulate supers [ib*4+s0, ib*4+s0+ns) -> dest rows s0*32..(s0+ns)*32
            if first:
                agg_ga[ib] = psg.tile([128, N], f32, tag="g", name="agg")
            ga = agg_ga[ib]
            for h in range(H):
                o = ga[s0 * 32:(s0 + ns) * 32, h * 32:h * 32 + 17]
                for k in range(8):
                    lhs = stv[:, k, ib * 4 + s0:ib * 4 + s0 + ns, :, h]
                    rhs = xr_mod[:, k * 68 + h * 17: k * 68 + (h + 1) * 17]
                    nc.tensor.matmul(o, lhs, rhs,
                                     start=(k == 0), stop=(k == 7),
                                     skip_group_check=True,
                                     tile_position=(0, s0 * 32))

        ofs = {}

        def agg_div(ib, r0, r1, first):
            ga = agg_ga[ib]
            if first:
                ofs[ib] = (outp.tile([128, HC], f32, tag="outf", name="outf"),
                           outp.tile([128, HC], f32, tag="outf2", name="outf2"))
            out_f, out_f2 = ofs[ib]
            for h in range(H):
                rz = work.tile([128, 1], f32, tag="rz", name="rz")
                nc.vector.reciprocal(rz[r0:r1, :],
                                     ga[r0:r1, h * 32 + 16:h * 32 + 17])
                nc.vector.tensor_scalar(out_f[r0:r1, h * 16:(h + 1) * 16],
                                        ga[r0:r1, h * 32:h * 32 + 16],
                                        rz[r0:r1, 0:1], None, Alu.mult)
            nc.vector.tensor_add(out_f2[r0:r1, :], out_f[r0:r1, :],
                                 biasb_t[r0:r1, :] if hasattr(biasb_t, 'rearrange') else biasb_t)
            dma(out[ib * 128 + r0:ib * 128 + r1, :], out_f2[r0:r1, :])

        for sup in range(NSUP):
            super_iter(sup)
            if sup == 5:
                agg_mms(0, 0, 4, True)
            elif sup == 6:
                agg_div(0, 0, 128, True)
                agg_mms(1, 0, 3, True)
        agg_div(1, 0, 96, True)
        agg_mms(1, 3, 1, False)
        agg_div(1, 96, 128, False)


def _get_program():
    if "nc" not in _CACHE:
        _CACHE["nc"] = _build_program()
    return _CACHE["nc"]


def kernel(x, adj, W_l, b_l, W_r, b_r, att, bias):
    global LAST_RESULTS
    from concourse.bass_utils import run_bass_kernel_spmd

    x = np.ascontiguousarray(np.asarray(x, dtype=np.float32))
    adj = np.ascontiguousarray(np.asarray(adj, dtype=np.float32))
    W_l = np.asarray(W_l, dtype=np.float32)
    b_l = np.asarray(b_l, dtype=np.float32)
    W_r = np.asarray(W_r, dtype=np.float32)
    b_r = np.asarray(b_r, dtype=np.float32)
    att = np.asarray(att, dtype=np.float32)
    bias = np.asarray(bias, dtype=np.float32)

    # host-side constant prep
    att2p8 = np.zeros((F, 8), np.float32)        # 0.8 * att, block diagonal x2
    for d in range(2):
        for h in range(H):
            att2p8[d * HC + h * C:(d * HC + (h + 1) * C), d * H + h] = 0.8 * att[h]
    attblk = np.zeros((HC, H), np.float32)       # att_blk
    for h in range(H):
        attblk[h * C:(h + 1) * C, h] = att[h]
    wrab = np.concatenate([W_r, W_l @ attblk], axis=1)   # [F, 68]
    e30 = np.zeros((128, 128), np.float32)       # E30[r, a*4+h] = (r%32 == a)
    r = np.arange(128)
    for a in range(32):
        for h in range(H):
            e30[r[r % 32 == a], a * 4 + h] = 1.0

    blob = np.zeros((128, BLOB_COLS), np.float32)

    def put(nm, arr):
        w = arr.shape[1]
        blob[:arr.shape[0], _OFF[nm]:_OFF[nm] + w] = arr

    put("wl", W_l)
    put("wr", W_r)
    put("wrab", wrab)
    put("ident", np.eye(128, dtype=np.float32))
    put("biasb", np.broadcast_to(bias + b_r, (128, HC)))
    put("att2p8", att2p8)
    put("blp2", np.tile(b_l, 2).reshape(128, 1))
    put("brp2", np.tile(b_r, 2).reshape(128, 1))
    put("e30", e30)

    in_maps = []
    for core in range(NCORES):
        b, blk = core // 4, core % 4
        i0 = blk * NI
        cblob = blob.copy()
        # xin[p, k*128+f] = x[b, k*128+p, f]
        cblob[:, _OFF["xb"]:_OFF["xb"] + 8 * F] = (
            x[b].reshape(8, 128, F).transpose(1, 0, 2).reshape(128, 8 * F))
        cblob[:, _OFF["xis"]:_OFF["xis"] + 2 * F] = (
            x[b, i0:i0 + NI].reshape(2, 128, F).transpose(1, 0, 2).reshape(128, 2 * F))
        adjsl = adj[b, i0:i0 + NI, :].copy()
        adjsl[np.arange(NI), i0 + np.arange(NI)] = 1.0   # self loops
        adjm30 = (MASKVAL * (adjsl - 1.0)).astype(np.float16)
        # adjm[p, ib*N + j] = adjm30[ib*128 + p, j]
        adjm = adjm30.reshape(2, 128, N).transpose(1, 0, 2).reshape(128, 2 * N).copy()
        in_maps.append({"blob": cblob, "adjs": adjm})

    nc = _get_program()
    res = run_bass_kernel_spmd(nc, in_maps, core_ids=list(range(NCORES)))
    LAST_RESULTS = res
    outp = np.zeros((B, N, HC), np.float32)
    for core in range(NCORES):
        b, blk = core // 4, core % 4
        outp[b, blk * NI:(blk + 1) * NI, :] = res.results[core]["out"]
    return outp


# revision 15
# speedup vs baseline: 1.0913x; 1.0633x over previous
# DenseGATv2Conv Trainium2 kernel.
#
# Math (per batch b):
#   xl = x @ W_l + b_l ; xr = x @ W_r + b_r            [N, H*C]
#   alpha[i,j,h] = sum_c att[h,c] * leaky_relu(xl[j,hc] + xr[i,hc], 0.2)
#   S = softmax_j(alpha masked by adj(+self loops))
#   out[i,hc] = sum_j S[i,j,h] * xr[j,hc] + bias
#
# Key identity used on device:
#   leaky_relu(z) = 0.2*z + 0.8*relu(z)
#   alpha[i,j,h] = 0.2*sl[j,h] + 0.2*sr[i,h] + 0.8*sum_c att[h,c]*relu(xl[j,hc]+xr[i,hc])
# where sl = xl @ att_blk, sr = xr @ att_blk are rank-1 in the (i,j) plane.
# In softmax over j the exp(0.2*sr[i,h]) factor cancels; exp(0.2*sl[j,h]) is
# folded multiplicatively into the aggregation operand.  Since sum_j Sbar = 1,
# the value bias b_r and the output bias fold into one final add.
#
# Device scheme: per pair of destination rows (16 pairs = 32 dest rows per
# "super"), DVE/Act/Pool build rp[(d,hc), j] = relu(xl[j,hc]+xr[i_d,hc]) f16.
# The score contraction uses rp as the matmul STATIONARY operand with a tiny
# [128, 8] att operand moving, so results land in PSUM already transposed:
# ps[j, (pair,d,h)].  The adjacency mask is an extra accumulating matmul
# adding -30 to masked entries (adjm30 = 30*(adj-1) f16 stationary, E30
# selection moving); one exp() per super then yields masked scores (f16
# underflow -> exact 0).  Softmax numerator+denominator come from one
# aggregation matmul per (head, k) with an esl column appended.
#
# All transposed operands (xbT, xisT, weights) are shipped pre-transposed
# from the host, so the device does no PE transposes at all.
#
# Sharding: 8 cores = (batch b in 0..1) x (4 blocks of 256 destination rows).

import numpy as np

B, N, F, H, C = 2, 1024, 128, 4, 16
HC = H * C
NCORES = 8
NI = 256          # destination rows per core
NPAIR = NI // 2   # 128 pairs of destination rows
NSUP = 8          # supers of 16 pairs (32 dest rows) each
NEG = 0.2
MASKVAL = 30.0

_CACHE = {}
LAST_RESULTS = None

# engine assignment for the 16 rp builds of each super:
# 'd' = DVE tensor_scalar, 'a' = Act activation, 'p' = Pool tensor_scalar
RP_SCHED = "ddddddddddddaapp"

# cr (f32r [128, 516]): xisT(256), wl2(128), wr(64), wrab(68)
CR_XIST, CR_WL2, CR_WR, CR_WRAB, CR_COLS = 0, 256, 384, 448, 516
# c16 (f16 [128, 136]): att2p8(8), e30(128)
C16_ATT8, C16_E30, C16_COLS = 0, 8, 136
# cf (f32 [128, 66]): biasb(64), blp2(1), brp2(1)
CF_BIASB, CF_BLP2, CF_BRP2, CF_COLS = 0, 64, 65, 66


def _build_program(debug=False):
    import concourse.bass as bass
    import concourse.mybir as mybir
    import concourse.tile as tile
    from concourse import bacc

    f32 = mybir.dt.float32
    f32r = mybir.dt.float32r
    f16 = mybir.dt.float16

    nc = bacc.Bacc(
        "TRN2",
        target_bir_lowering=False,
        debug=False,
        enable_asserts=False,
        num_devices=NCORES,
    )

    # ---- DRAM I/O ----
    cf = nc.dram_tensor("cf", [128, CF_COLS], f32, kind="ExternalInput").ap()
    cr = nc.dram_tensor("cr", [F, CR_COLS], f32r, kind="ExternalInput").ap()
    xbT = nc.dram_tensor("xbT", [F, N], f32r, kind="ExternalInput").ap()
    c16 = nc.dram_tensor("c16", [128, C16_COLS], f16, kind="ExternalInput").ap()
    adjs = nc.dram_tensor("adjs", [128, 2 * N], f16, kind="ExternalInput").ap()
    out = nc.dram_tensor("out", [NI, HC], f32, kind="ExternalOutput").ap()

    with tile.TileContext(nc) as tc:
        _body(tc, nc, mybir, bass, f32, f32r, f16, cf, cr, xbT, c16, adjs, out)

    nc.compile()
    return nc


def _body(tc, nc, mybir, bass, f32, f32r, f16, cf, cr, xbT, c16, adjs, out):
    from contextlib import ExitStack
    Alu = mybir.AluOpType
    Act = mybir.ActivationFunctionType
    ctx = ExitStack()
    with ctx:
        consts = ctx.enter_context(tc.tile_pool(name="consts", bufs=1))
        work = ctx.enter_context(tc.tile_pool(name="work", bufs=1))
        rp_pool = ctx.enter_context(tc.tile_pool(name="rp", bufs=2))
        outp = ctx.enter_context(tc.tile_pool(name="outp", bufs=2))
        psg = ctx.enter_context(tc.tile_pool(name="psg", bufs=1, space="PSUM"))
        pss = ctx.enter_context(tc.tile_pool(name="pss", bufs=2, space="PSUM"))
        pst = ctx.enter_context(tc.tile_pool(name="pst", bufs=2, space="PSUM"))

        dma = nc.sync.dma_start

        # ---------- load inputs ----------
        cf_t = consts.tile([128, CF_COLS], f32, tag="cf")
        cr_t = consts.tile([F, CR_COLS], f32r, tag="cr")
        xbT_t = consts.tile([F, N], f32r, tag="xbT")
        c16_t = consts.tile([128, C16_COLS], f16, tag="c16")
        adjm = consts.tile([128, 2 * N], f16, tag="adjm")  # 30*(adj-1), [i128][ib*N+j]
        dma(cf_t[:], cf)
        dma(cr_t[:], cr)
        dma(xbT_t[:], xbT)
        dma(c16_t[:], c16)
        dma(adjm[:], adjs)

        xisT = cr_t[:, CR_XIST:CR_XIST + NI]
        wl2_r = cr_t[:, CR_WL2:CR_WL2 + 128]
        wr_r = cr_t[:, CR_WR:CR_WR + HC]
        wrab_r = cr_t[:, CR_WRAB:CR_WRAB + HC + H]
        att8_r = c16_t[:, C16_ATT8:C16_ATT8 + 8]
        e30_16 = c16_t[:, C16_E30:C16_E30 + 128]
        biasb_t = cf_t[:, CF_BIASB:CF_BIASB + HC]
        blp2_t = cf_t[:, CF_BLP2:CF_BLP2 + 1]
        brp_t = cf_t[0:HC, CF_BRP2:CF_BRP2 + 1]

        # ---------- xrsT / xrp (critical path to rp) ----------
        xrsT = consts.tile([HC, NI], f32, tag="xrsT")
        g3 = psg.tile([128, N], f32, tag="g")
        pj3 = g3[0:HC, 0:NI]
        nc.tensor.matmul(pj3, wr_r, xisT, start=True, stop=True)
        nc.scalar.activation(xrsT[:], pj3, Act.Identity, bias=brp_t, scale=1.0)
        xrp = consts.tile([128, NPAIR], f32, tag="xrp")
        ev = xrsT[:].rearrange("p (a two) -> p a two", two=2)
        nc.vector.tensor_copy(xrp[0:HC, :], ev[:, :, 0])
        nc.vector.tensor_copy(xrp[HC:128, :], ev[:, :, 1])

        # ---------- xl2T = [(x@W_l + b_l)^T ; same] via [W_l|W_l] ----------
        xl2T = consts.tile([128, N], f16, tag="xl2T")
        gp = psg.tile([128, N], f32, tag="g")
        for half in range(2):
            s = slice(half * 512, (half + 1) * 512)
            nc.tensor.matmul(gp[:, s], wl2_r, xbT_t[:, s], start=True, stop=True)
        nc.scalar.activation(xl2T[:], gp[:], Act.Identity, bias=blp2_t, scale=1.0)

        # ---------- xr_mod: aggregation moving operand (natural layout) ----------
        # xr_mod[j, k*68 + h*17 + c] = xr_nob[j,hc]*esl[j,h] (c<16); c=16: esl[j,h]
        # xr_nob excludes b_r (folded into the output bias since sum_j Sbar = 1);
        # esl = exp(0.2 * x @ (W_l @ att_blk)) (the b_l@att_blk factor cancels).
        xr_mod = consts.tile([128, 8 * 68], f16, tag="xrmod")
        xrmv = xr_mod[:].rearrange("p (k h c) -> p k h c", k=8, h=H)

        def build_xr_mod_k(k):
            pkt = pst.tile([128, 128], f32, tag="pt", name="pk")
            pk = pkt[:, 0:HC + H]
            nc.tensor.matmul(pk, xbT_t[:, k * 128:(k + 1) * 128], wrab_r,
                             start=True, stop=True)
            esl4 = work.tile([128, 8 * H], f16, tag="esl4", name="esl4")
            xr16 = work.tile([128, 8 * HC], f16, tag="xr16", name="xr16")
            nc.scalar.activation(esl4[:, k * H:(k + 1) * H],
                                 pkt[:, HC:HC + H], Act.Exp, scale=NEG)
            nc.scalar.copy(xr16[:, k * HC:(k + 1) * HC], pkt[:, 0:HC])
            nc.gpsimd.tensor_copy(xrmv[:, k, :, 16], esl4[:, k * H:(k + 1) * H])
            rep = esl4[:, k * H:(k + 1) * H].rearrange(
                "p (h one) -> p h one", one=1).broadcast_to([128, H, 16])
            srcx = xr16[:, k * HC:(k + 1) * HC].rearrange("p (h c) -> p h c", h=H)
            nc.gpsimd.tensor_tensor(xrmv[:, k, :, 0:16], srcx, rep, Alu.mult)

        # ---------- main streaming loop ----------
        # st_all[j, k*1024 + s*128 + a*4 + h], a = local dest (2*p+d) in super s
        # (k-major so the aggregation lhsT slice has ONE contiguous free dim)
        st_all = consts.tile([128, NSUP * N], f16, tag="stall")
        stv = st_all[:].rearrange("p (k s a h) -> p k s a h", k=8, s=NSUP, h=H)

        def super_iter(sup):
            ib, s4 = sup // 4, sup % 4
            if 1 <= sup <= 4:
                build_xr_mod_k(2 * (sup - 1))
                build_xr_mod_k(2 * (sup - 1) + 1)
            rp = rp_pool.tile([128, 16, 1024], f16, tag="rp")
            for p in range(16):
                gp_ = sup * 16 + p
                kind = RP_SCHED[p]
                if kind == "a":
                    nc.scalar.activation(rp[:, p, :], xl2T[:], Act.Relu,
                                         bias=xrp[:, gp_:gp_ + 1], scale=1.0)
                elif kind == "p":
                    nc.gpsimd.tensor_scalar(rp[:, p, :], xl2T[:],
                                            xrp[:, gp_:gp_ + 1],
                                            0.0, Alu.add, Alu.max)
                else:
                    nc.vector.tensor_scalar(rp[:, p, :], xl2T[:],
                                            xrp[:, gp_:gp_ + 1],
                                            0.0, Alu.add, Alu.max)
            ps = pss.tile([128, 8, 128], f32, tag="sc")
            for k in range(8):
                # mask matmul: adds 30*(adj-1) (0 or -30) to every (j, a, h)
                nc.tensor.matmul(ps[:, k, :],
                                 adjm[s4 * 32:(s4 + 1) * 32,
                                      ib * N + k * 128: ib * N + (k + 1) * 128],
                                 e30_16[s4 * 32:(s4 + 1) * 32, :],
                                 start=True, stop=False, skip_group_check=True,
                                 tile_position=(s4 * 32, 0))
                for p in range(16):
                    nc.tensor.matmul(ps[:, k, p * 8:(p + 1) * 8],
                                     rp[:, p, k * 128:(k + 1) * 128],
                                     att8_r,
                                     start=False, stop=(p == 15),
                                     skip_group_check=True)
            nc.scalar.activation(stv[:, :, sup, :, :],
                                 ps[:].rearrange("p k (a h) -> p k a h", h=H),
                                 Act.Exp)

        # ---------- aggregation ----------
        # agg psum: one [128, 1024] tile per ib; head h uses cols h*32..h*32+17
        agg_ga = {}

        def agg_mms(ib, s0, ns, first):
            # accumulate supers [ib*4+s0, ib*4+s0+ns) -> dest rows s0*32..(s0+ns)*32
            if first:
                agg_ga[ib] = psg.tile([128, N], f32, tag="g", name="agg")
            ga = agg_ga[ib]
            for h in range(H):
                o = ga[s0 * 32:(s0 + ns) * 32, h * 32:h * 32 + 17]
                for k in range(8):
                    lhs = stv[:, k, ib * 4 + s0:ib * 4 + s0 + ns, :, h]
                    rhs = xr_mod[:, k * 68 + h * 17: k * 68 + (h + 1) * 17]
                    nc.tensor.matmul(o, lhs, rhs,
                                     start=(k == 0), stop=(k == 7),
                                     skip_group_check=True,
                                     tile_position=(0, s0 * 32))

        ofs = {}

        def agg_div(ib, r0, r1, first):
            ga = agg_ga[ib]
            if first:
                ofs[ib] = (outp.tile([128, HC], f32, tag="outf", name="outf"),
                           outp.tile([128, HC], f32, tag="outf2", name="outf2"))
            out_f, out_f2 = ofs[ib]
            for h in range(H):
                rz = work.tile([128, 1], f32, tag="rz", name="rz")
                nc.vector.reciprocal(rz[r0:r1, :],
                                     ga[r0:r1, h * 32 + 16:h * 32 + 17])
                nc.vector.tensor_scalar(out_f[r0:r1, h * 16:(h + 1) * 16],
                                        ga[r0:r1, h * 32:h * 32 + 16],
                                        rz[r0:r1, 0:1], None, Alu.mult)
            nc.vector.tensor_add(out_f2[r0:r1, :], out_f[r0:r1, :],
                                 biasb_t[r0:r1, :])
            dma(out[ib * 128 + r0:ib * 128 + r1, :], out_f2[r0:r1, :])

        for sup in range(NSUP):
            super_iter(sup)
            if sup == 5:
                agg_mms(0, 0, 4, True)
            elif sup == 6:
                agg_div(0, 0, 128, True)
                agg_mms(1, 0, 3, True)
        agg_div(1, 0, 96, True)
        agg_mms(1, 3, 1, False)
        agg_div(1, 96, 128, False)


def _get_program():
    if "nc" not in _CACHE:
        _CACHE["nc"] = _build_program()
    return _CACHE["nc"]


def kernel(x, adj, W_l, b_l, W_r, b_r, att, bias):
    global LAST_RESULTS
    from concourse.bass_utils import run_bass_kernel_spmd

    x = np.ascontiguousarray(np.asarray(x, dtype=np.float32))
    adj = np.ascontiguousarray(np.asarray(adj, dtype=np.float32))
    W_l = np.asarray(W_l, dtype=np.float32)
    b_l = np.asarray(b_l, dtype=np.float32)
    W_r = np.asarray(W_r, dtype=np.float32)
    b_r = np.asarray(b_r, dtype=np.float32)
    att = np.asarray(att, dtype=np.float32)
    bias = np.asarray(bias, dtype=np.float32)

    # host-side constant prep
    att2p8 = np.zeros((F, 8), np.float32)        # 0.8 * att, block diagonal x2
    for d in range(2):
        for h in range(H):
            att2p8[d * HC + h * C:(d * HC + (h + 1) * C), d * H + h] = 0.8 * att[h]
    attblk = np.zeros((HC, H), np.float32)       # att_blk
    for h in range(H):
        attblk[h * C:(h + 1) * C, h] = att[h]
    wrab = np.concatenate([W_r, W_l @ attblk], axis=1)   # [F, 68]
    e30 = np.zeros((128, 128), np.float32)       # E30[r, a*4+h] = (r%32 == a)
    r = np.arange(128)
    for a in range(32):
        for h in range(H):
            e30[r[r % 32 == a], a * 4 + h] = 1.0

    c16 = np.zeros((128, C16_COLS), np.float16)
    c16[:, C16_ATT8:C16_ATT8 + 8] = att2p8
    c16[:, C16_E30:C16_E30 + 128] = e30
    cf = np.zeros((128, CF_COLS), np.float32)
    cf[:, CF_BIASB:CF_BIASB + HC] = np.broadcast_to(bias + b_r, (128, HC))
    cf[:, CF_BLP2] = np.tile(b_l, 2)
    cf[:, CF_BRP2] = np.tile(b_r, 2)

    wl2 = np.concatenate([W_l, W_l], axis=1)

    in_maps = []
    for core in range(NCORES):
        b, blk = core // 4, core % 4
        i0 = blk * NI
        cr_arr = np.zeros((F, CR_COLS), np.float32)
        cr_arr[:, CR_XIST:CR_XIST + NI] = x[b, i0:i0 + NI].T
        cr_arr[:, CR_WL2:CR_WL2 + 128] = wl2
        cr_arr[:, CR_WR:CR_WR + HC] = W_r
        cr_arr[:, CR_WRAB:CR_WRAB + HC + H] = wrab
        xbT_arr = np.ascontiguousarray(x[b].T)
        adjsl = adj[b, i0:i0 + NI, :].copy()
        adjsl[np.arange(NI), i0 + np.arange(NI)] = 1.0   # self loops
        adjm30 = (MASKVAL * (adjsl - 1.0)).astype(np.float16)
        # adjm[p, ib*N + j] = adjm30[ib*128 + p, j]
        adjm = adjm30.reshape(2, 128, N).transpose(1, 0, 2).reshape(128, 2 * N).copy()
        in_maps.append({"cf": cf, "cr": cr_arr, "xbT": xbT_arr, "c16": c16,
                        "adjs": adjm})

    nc = _get_program()
    res = run_bass_kernel_spmd(nc, in_maps, core_ids=list(range(NCORES)))
    LAST_RESULTS = res
    outp = np.zeros((B, N, HC), np.float32)
    for core in range(NCORES):
        b, blk = core // 4, core % 4
        outp[b, blk * NI:(blk + 1) * NI, :] = res.results[core]["out"]
    return outp
